# revision 2
# baseline (speedup 1.0000x reference)
"""Trainium2 Bass kernel for FFT-conv1d (= valid cross-correlation conv1d).

Reference computes, for x[N=64, C=64, W=4096], w[F=64, C=64, WW=16], b[F=64]:
    out[n, f, t] = sum_{c, j} x[n, c, t + j] * w[f, c, j] + b[f],  t in [0, 4081)

Strategy (v3, 2-parallel fast-FIR / Karatsuba + phase-block layout):
  - Data-parallel: shard N across 8 NeuronCores (8 samples per core).
  - Split outputs and taps by parity:  u[m]=x[2m], v[m]=x[2m+1],
    g[i]=h[2i], k[i]=h[2i+1] (length-8 subfilters).  With
      A = g*u,  B = k*v,  C = (g+k)*(v + u(+1))
    the outputs are   y[2m] = A[m] + B[m],  y[2m+1] = C[m] - A[m+1] - B[m].
    3 length-8 correlations instead of 4  ->  25% fewer TensorE cycles.
  - Each correlation runs as 4 accumulating K=128 matmuls: SBUF rows 0-63
    hold phase[c, m], rows 64-127 hold phase[c, m+1] (block layout; all
    rhs access patterns are contiguous).  Two samples occupy PE column
    groups 0/64 and stream concurrently.
  - u/v/s phases are precomputed on host (s = v + u(+1)); the row 64-127
    shifted halves are built by SBUF->SBUF shift DMAs (pairs 1-3) or
    loaded straight from HBM with +1 element offset (pair 0, so the
    first matmul never waits on a 2-hop load->shift chain).
  - Combines: ACT evacuates B with fused bias (Bs = B + b); DVE does
    y_even = A + Bs, t = C - Bs, y_odd = (t + 2b) - A' -- 3 DVE ops +
    1 ACT op per 1022 outputs, written bf16 (host upcasts to fp32).
"""

import numpy as np

N, C, W = 64, 64, 4096
F, WW = 64, 16
OUT_W = W - WW + 1  # 4081
N_CORES = 8
NPC = N // N_CORES  # samples per core = 8
PW = 2056           # padded phase width (max col read = 1533+6+511 = 2050)
M0S = (0, 511, 1022, 1533)   # m-tile bases (511-stride, 512-wide banks)
CH = (0, 1040, 2056)         # load/shift chunk boundaries in phase cols

_CACHE = {}


def _build_nc():
    from contextlib import ExitStack

    import concourse.bacc as bacc
    import concourse.mybir as mybir
    import concourse.tile as tile

    f32 = mybir.dt.float32
    bf16 = mybir.dt.bfloat16
    ADD = mybir.AluOpType.add
    SUB = mybir.AluOpType.subtract

    nc = bacc.Bacc(
        "TRN2", target_bir_lowering=False, debug=False, num_devices=N_CORES
    )
    xu_d = nc.dram_tensor("xu", [NPC, C, PW], bf16, kind="ExternalInput").ap()
    xv_d = nc.dram_tensor("xv", [NPC, C, PW], bf16, kind="ExternalInput").ap()
    ss_d = nc.dram_tensor("ss", [NPC, 128, PW], bf16, kind="ExternalInput").ap()
    w_d = nc.dram_tensor("wstk", [128, 768], bf16, kind="ExternalInput").ap()
    b1_d = nc.dram_tensor("bias1", [128, 1], f32, kind="ExternalInput").ap()
    b2_d = nc.dram_tensor("bias2", [128, 1], f32, kind="ExternalInput").ap()
    o_d = nc.dram_tensor("out", [NPC, F, OUT_W], bf16, kind="ExternalOutput").ap()

    with tile.TileContext(nc) as tc:
        with ExitStack() as ctx:
            consts = ctx.enter_context(tc.tile_pool(name="consts", bufs=1))
            xpool = ctx.enter_context(tc.tile_pool(name="xs", bufs=12))
            opool = ctx.enter_context(tc.tile_pool(name="osb", bufs=3))
            tpool = ctx.enter_context(tc.tile_pool(name="tmp", bufs=4))
            pspool = ctx.enter_context(
                tc.tile_pool(name="ps", bufs=8, space="PSUM")
            )

            wsb = consts.tile([128, 768], bf16)
            nc.gpsimd.dma_start(out=wsb[:, :], in_=w_d[:, :])
            b1sb = consts.tile([128, 1], f32)
            nc.gpsimd.dma_start(out=b1sb[:, :], in_=b1_d[:, :])
            b2sb = consts.tile([128, 1], f32)
            nc.gpsimd.dma_start(out=b2sb[:, :], in_=b2_d[:, :])

            tiles = {}

            def emit_loads(p):
                # sample 0 of the pair loads on the sync HWDGE queue,
                # sample 1 on the scalar HWDGE queue; shifted row-64
                # halves come straight from HBM (+1 col) for pair 0 and
                # via SBUF->SBUF shifts on the gpsimd SWDGE queue after.
                trio = []
                for s, eng in ((0, nc.sync), (1, nc.scalar)):
                    n = 2 * p + s
                    smp = []
                    for src in (xu_d, xv_d, ss_d):
                        xt = xpool.tile([128, PW], bf16)
                        full = src is ss_d
                        for ci in range(2):
                            lo, hi = CH[ci], CH[ci + 1]
                            eng.dma_start(
                                out=xt[0 : (128 if full else 64), lo:hi],
                                in_=src[n, :, lo:hi],
                            )
                        if not full:
                            if p == 0:
                                # head loads: same HBM rows, +1 col offset
                                for ci in range(2):
                                    lo, hi = CH[ci], CH[ci + 1]
                                    hi2 = min(hi, PW - 1)
                                    eng.dma_start(
                                        out=xt[64:128, lo:hi2],
                                        in_=src[n, :, lo + 1 : hi2 + 1],
                                    )
                            else:
                                nc.gpsimd.dma_start(
                                    out=xt[64:128, 0 : CH[1] - 1],
                                    in_=xt[0:64, 1 : CH[1]],
                                )
                                nc.gpsimd.dma_start(
                                    out=xt[64:128, CH[1] - 1 : PW - 5],
                                    in_=xt[0:64, CH[1] : PW - 4],
                                )
                        smp.append(xt)
                    trio.append(smp)
                tiles[p] = trio

            def emit_compute(p):
                trio = tiles.pop(p)
                osb = opool.tile([128, 4090], bf16)
                for tt, m0 in enumerate(M0S):
                    banks = []
                    for d in range(3):
                        ps = pspool.tile([128, 512], f32)
                        for a in range(4):
                            blk = d * 4 + a
                            for s in range(2):
                                nc.tensor.matmul(
                                    ps[64 * s : 64 * (s + 1), :],
                                    lhsT=wsb[:, blk * 64 : (blk + 1) * 64],
                                    rhs=trio[s][d][:, m0 + 2 * a : m0 + 2 * a + 512],
                                    start=(a == 0),
                                    stop=(a == 3),
                                )
                        banks.append(ps)
                    psA, psB, psC = banks
                    tB = tpool.tile([128, 512], f32)
                    nc.scalar.add(tB[:, :], psB[:, :], b1sb[:, 0:1])
                    nc.vector.tensor_add(
                        osb[:, 2 * m0 : 2 * m0 + 1022 : 2],
                        psA[:, 0:511],
                        tB[:, 0:511],
                    )
                    tT = tpool.tile([128, 511], f32)
                    nc.vector.tensor_sub(tT[:, :], psC[:, 0:511], tB[:, 0:511])
                    nc.vector.scalar_tensor_tensor(
                        osb[:, 2 * m0 + 1 : 2 * m0 + 1023 : 2],
                        tT[:, :],
                        b2sb[:, 0:1],
                        psA[:, 1:512],
                        ADD,
                        SUB,
                    )
                    lo = 2 * m0
                    hi = min(lo + 1022, OUT_W)
                    if p == 3 and tt == 3:
                        eng = nc.sync
                    elif p == 3 and tt == 2:
                        eng = nc.scalar
                    else:
                        eng = nc.gpsimd
                    eng.dma_start(
                        out=o_d[2 * p : 2 * p + 2].flatten_outer_dims()[:, lo:hi],
                        in_=osb[:, lo:hi],
                    )

            emit_loads(0)
            emit_loads(1)
            for p in range(4):
                emit_compute(p)
                if p + 2 <= 3:
                    emit_loads(p + 2)

    nc.compile()
    return nc


def _get_nc():
    if "nc" not in _CACHE:
        _CACHE["nc"] = _build_nc()
    return _CACHE["nc"]


def _host_prep(x, w, b):
    """Phase-split x, build s = v + u(+1), pack subfilter weights."""
    import ml_dtypes

    bf16 = ml_dtypes.bfloat16
    n = x.shape[0]
    u = np.zeros((n, C, PW), dtype=bf16)
    u[:, :, :2048] = x[:, :, 0::2]
    v = np.zeros((n, C, PW), dtype=bf16)
    v[:, :, :2048] = x[:, :, 1::2]
    sf = np.zeros((n, C, PW), dtype=bf16)
    sf[:, :, :2047] = x[:, :, 1:4094:2] + x[:, :, 2:4095:2]
    ss = np.zeros((n, 128, PW), dtype=bf16)
    ss[:, 0:64] = sf
    ss[:, 64:128, : PW - 1] = sf[:, :, 1:]

    # wstk[row, (d*4+a)*64 + f]: rows 0-63 tap 4a+{0,1,d:0+1}, rows 64-127
    # tap 4a+{2,3,2+3} for d = A,B,C
    wA0 = w[:, :, 0::4].transpose(1, 2, 0).reshape(C, 256)   # [c, a*64+f]
    wA1 = w[:, :, 2::4].transpose(1, 2, 0).reshape(C, 256)
    wB0 = w[:, :, 1::4].transpose(1, 2, 0).reshape(C, 256)
    wB1 = w[:, :, 3::4].transpose(1, 2, 0).reshape(C, 256)
    wstk = np.zeros((128, 768), dtype=np.float32)
    wstk[0:64, 0:256] = wA0
    wstk[64:128, 0:256] = wA1
    wstk[0:64, 256:512] = wB0
    wstk[64:128, 256:512] = wB1
    wstk[0:64, 512:768] = wA0 + wB0
    wstk[64:128, 512:768] = wA1 + wB1
    wstk = np.ascontiguousarray(wstk.astype(bf16))
    b1 = np.ascontiguousarray(np.concatenate([b, b]).astype(np.float32).reshape(128, 1))
    b2 = np.ascontiguousarray((2.0 * b1).astype(np.float32))
    return u, v, ss, wstk, b1, b2


def _make_in_maps(x, w, b):
    u, v, ss, wstk, b1, b2 = _host_prep(x, w, b)
    return [
        {
            "xu": np.ascontiguousarray(u[i * NPC : (i + 1) * NPC]),
            "xv": np.ascontiguousarray(v[i * NPC : (i + 1) * NPC]),
            "ss": np.ascontiguousarray(ss[i * NPC : (i + 1) * NPC]),
            "wstk": wstk,
            "bias1": b1,
            "bias2": b2,
        }
        for i in range(N_CORES)
    ]


def kernel(x, w, b):
    from concourse.bass_utils import run_bass_kernel_spmd

    x = np.asarray(x, dtype=np.float32)
    w = np.asarray(w, dtype=np.float32)
    b = np.asarray(b, dtype=np.float32)
    assert x.shape == (N, C, W) and w.shape == (F, C, WW) and b.shape == (F,)

    nc = _get_nc()
    in_maps = _make_in_maps(x, w, b)
    res = run_bass_kernel_spmd(nc, in_maps, core_ids=list(range(N_CORES)))
    out = np.concatenate([np.asarray(r["out"]) for r in res.results], axis=0)
    return out.astype(np.float32)


# revision 9
# speedup vs baseline: 1.0047x; 1.0047x over previous
"""Trainium2 Bass kernel for FFT-conv1d (= valid cross-correlation conv1d).

Reference computes, for x[N=64, C=64, W=4096], w[F=64, C=64, WW=16], b[F=64]:
    out[n, f, t] = sum_{c, j} x[n, c, t + j] * w[f, c, j] + b[f],  t in [0, 4081)

Strategy (v3, 2-parallel fast-FIR / Karatsuba + phase-block layout):
  - Data-parallel: shard N across 8 NeuronCores (8 samples per core).
  - Split outputs and taps by parity:  u[m]=x[2m], v[m]=x[2m+1],
    g[i]=h[2i], k[i]=h[2i+1] (length-8 subfilters).  With
      A = g*u,  B = k*v,  C = (g+k)*(v + u(+1))
    the outputs are   y[2m] = A[m] + B[m],  y[2m+1] = C[m] - A[m+1] - B[m].
    3 length-8 correlations instead of 4  ->  25% fewer TensorE cycles.
  - Each correlation runs as 4 accumulating K=128 matmuls: SBUF rows 0-63
    hold phase[c, m], rows 64-127 hold phase[c, m+1] (block layout; all
    rhs access patterns are contiguous).  Two samples occupy PE column
    groups 0/64 and stream concurrently.
  - u/v/s phases are precomputed on host (s = v + u(+1)); the row 64-127
    shifted halves are built by SBUF->SBUF shift DMAs (pairs 1-3) or
    loaded straight from HBM with +1 element offset (pair 0, so the
    first matmul never waits on a 2-hop load->shift chain).
  - Combines: ACT evacuates B with fused bias (Bs = B + b); DVE does
    y_even = A + Bs, t = C - Bs, y_odd = (t + 2b) - A' -- 3 DVE ops +
    1 ACT op per 1022 outputs, written bf16 (host upcasts to fp32).
"""

import numpy as np

N, C, W = 64, 64, 4096
F, WW = 64, 16
OUT_W = W - WW + 1  # 4081
N_CORES = 8
NPC = N // N_CORES  # samples per core = 8
PW = 2056           # padded phase width (max col read = 1533+6+511 = 2050)
M0S = (0, 511, 1022, 1533)   # m-tile bases (511-stride, 512-wide banks)
CH = (0, 1040, 2056)         # load/shift chunk boundaries in phase cols

_CACHE = {}


def _build_nc():
    from contextlib import ExitStack

    import concourse.bacc as bacc
    import concourse.mybir as mybir
    import concourse.tile as tile

    f32 = mybir.dt.float32
    bf16 = mybir.dt.bfloat16
    ADD = mybir.AluOpType.add
    SUB = mybir.AluOpType.subtract

    nc = bacc.Bacc(
        "TRN2", target_bir_lowering=False, debug=False, num_devices=N_CORES
    )
    xu_d = nc.dram_tensor("xu", [NPC, C, PW], bf16, kind="ExternalInput").ap()
    xv_d = nc.dram_tensor("xv", [NPC, C, PW], bf16, kind="ExternalInput").ap()
    ss_d = nc.dram_tensor("ss", [NPC, 128, PW], bf16, kind="ExternalInput").ap()
    w_d = nc.dram_tensor("wstk", [128, 768], bf16, kind="ExternalInput").ap()
    b1_d = nc.dram_tensor("bias1", [128, 1], f32, kind="ExternalInput").ap()
    b2_d = nc.dram_tensor("bias2", [128, 1], f32, kind="ExternalInput").ap()
    o_d = nc.dram_tensor("out", [NPC, F, OUT_W], bf16, kind="ExternalOutput").ap()

    with tile.TileContext(nc) as tc:
        with ExitStack() as ctx:
            consts = ctx.enter_context(tc.tile_pool(name="consts", bufs=1))
            xpool = ctx.enter_context(tc.tile_pool(name="xs", bufs=4))
            opool = ctx.enter_context(tc.tile_pool(name="osb", bufs=3))
            tpool = ctx.enter_context(tc.tile_pool(name="tmp", bufs=4))
            pspool = ctx.enter_context(
                tc.tile_pool(name="ps", bufs=7, space="PSUM")
            )

            wsb = consts.tile([128, 768], bf16)
            nc.gpsimd.dma_start(out=wsb[:, :], in_=w_d[:, :])
            b1sb = consts.tile([128, 1], f32)
            nc.gpsimd.dma_start(out=b1sb[:, :], in_=b1_d[:, :])
            b2sb = consts.tile([128, 1], f32)
            nc.gpsimd.dma_start(out=b2sb[:, :], in_=b2_d[:, :])

            # HAM warm-up: keep the PE busy during the load phase so the
            # clock gate is at 8/8 before the first real matmul arrives.
            wrm = consts.tile([128, 32], bf16)
            nc.vector.memset(wrm[:, :], 0.0)
            pswm = pspool.tile([128, 512], f32, name="pswm", bufs=1)
            for _ in range(44):
                nc.tensor.matmul(
                    pswm[0:32, 0:32], lhsT=wrm[:, 0:32], rhs=wrm[:, :],
                    start=True, stop=True,
                )

            tiles = {}

            def emit_loads(p):
                # loads ride the sync HWDGE queue (pair 0's second sample
                # goes on scalar for a faster start), issued pair-major in
                # consumption order; shifted row-64 halves come straight
                # from HBM (+1 col) for pair 0 and via SBUF->SBUF shifts on
                # the (otherwise idle) gpsimd SWDGE queue for pairs 1-3.
                trio = [
                    [
                        xpool.tile([128, PW], bf16, name=f"x{s}{di}")
                        for di in range(3)
                    ]
                    for s in range(2)
                ]
                srcs = (xu_d, xv_d, ss_d)
                for s in range(2):
                    eng = nc.scalar if (p == 0 and s == 1) else nc.sync
                    n = 2 * p + s
                    for di, src in enumerate(srcs):
                        full = di == 2
                        xt = trio[s][di]
                        eng.dma_start(
                            out=xt[0 : (128 if full else 64), :],
                            in_=src[n, :, :],
                        )
                        if not full and p == 0:
                            eng.dma_start(
                                out=xt[64:128, 0 : PW - 1],
                                in_=src[n, :, 1:PW],
                            )
                if p > 0:
                    for di in range(2):
                        for s in range(2):
                            xt = trio[s][di]
                            nc.gpsimd.dma_start(
                                out=xt[64:128, 0 : PW - 5],
                                in_=xt[0:64, 1 : PW - 4],
                            )
                tiles[p] = trio

            def emit_compute(p):
                trio = tiles.pop(p)
                osb = opool.tile([128, 4090], bf16)
                for tt, m0 in enumerate(M0S):
                    banks = []
                    for d in range(3):
                        ps = pspool.tile([128, 512], f32, name="ps")
                        for a in range(4):
                            blk = d * 4 + a
                            for s in range(2):
                                nc.tensor.matmul(
                                    ps[64 * s : 64 * (s + 1), :],
                                    lhsT=wsb[:, blk * 64 : (blk + 1) * 64],
                                    rhs=trio[s][d][:, m0 + 2 * a : m0 + 2 * a + 512],
                                    start=(a == 0),
                                    stop=(a == 3),
                                )
                        banks.append(ps)
                    psA, psB, psC = banks
                    tB = tpool.tile([128, 512], f32)
                    nc.scalar.add(tB[:, :], psB[:, :], b1sb[:, 0:1])
                    nc.vector.tensor_add(
                        osb[:, 2 * m0 : 2 * m0 + 1022 : 2],
                        psA[:, 0:511],
                        tB[:, 0:511],
                    )
                    tT = tpool.tile([128, 511], f32)
                    nc.vector.tensor_sub(tT[:, :], psC[:, 0:511], tB[:, 0:511])
                    nc.vector.scalar_tensor_tensor(
                        osb[:, 2 * m0 + 1 : 2 * m0 + 1023 : 2],
                        tT[:, :],
                        b2sb[:, 0:1],
                        psA[:, 1:512],
                        ADD,
                        SUB,
                    )
                    lo = 2 * m0
                    hi = min(lo + 1022, OUT_W)
                    # stores interleave with the ACT bias-adds on the scalar
                    # HWDGE queue (deps resolve in FIFO order); the last
                    # pair's tail stores go on the by-then-idle sync queue
                    eng = nc.sync if (p == 3 and tt >= 2) else nc.scalar
                    eng.dma_start(
                        out=o_d[2 * p : 2 * p + 2].flatten_outer_dims()[:, lo:hi],
                        in_=osb[:, lo:hi],
                    )

            for p in range(4):
                emit_loads(p)
            for p in range(4):
                emit_compute(p)

    nc.compile()
    return nc


def _get_nc():
    if "nc" not in _CACHE:
        _CACHE["nc"] = _build_nc()
    return _CACHE["nc"]


def _host_prep(x, w, b):
    """Phase-split x, build s = v + u(+1), pack subfilter weights."""
    import ml_dtypes

    bf16 = ml_dtypes.bfloat16
    n = x.shape[0]
    u = np.zeros((n, C, PW), dtype=bf16)
    u[:, :, :2048] = x[:, :, 0::2]
    v = np.zeros((n, C, PW), dtype=bf16)
    v[:, :, :2048] = x[:, :, 1::2]
    sf = np.zeros((n, C, PW), dtype=bf16)
    sf[:, :, :2047] = x[:, :, 1:4094:2] + x[:, :, 2:4095:2]
    ss = np.zeros((n, 128, PW), dtype=bf16)
    ss[:, 0:64] = sf
    ss[:, 64:128, : PW - 1] = sf[:, :, 1:]

    # wstk[row, (d*4+a)*64 + f]: rows 0-63 tap 4a+{0,1,d:0+1}, rows 64-127
    # tap 4a+{2,3,2+3} for d = A,B,C
    wA0 = w[:, :, 0::4].transpose(1, 2, 0).reshape(C, 256)   # [c, a*64+f]
    wA1 = w[:, :, 2::4].transpose(1, 2, 0).reshape(C, 256)
    wB0 = w[:, :, 1::4].transpose(1, 2, 0).reshape(C, 256)
    wB1 = w[:, :, 3::4].transpose(1, 2, 0).reshape(C, 256)
    wstk = np.zeros((128, 768), dtype=np.float32)
    wstk[0:64, 0:256] = wA0
    wstk[64:128, 0:256] = wA1
    wstk[0:64, 256:512] = wB0
    wstk[64:128, 256:512] = wB1
    wstk[0:64, 512:768] = wA0 + wB0
    wstk[64:128, 512:768] = wA1 + wB1
    wstk = np.ascontiguousarray(wstk.astype(bf16))
    b1 = np.ascontiguousarray(np.concatenate([b, b]).astype(np.float32).reshape(128, 1))
    b2 = np.ascontiguousarray((2.0 * b1).astype(np.float32))
    return u, v, ss, wstk, b1, b2


def _make_in_maps(x, w, b):
    u, v, ss, wstk, b1, b2 = _host_prep(x, w, b)
    return [
        {
            "xu": np.ascontiguousarray(u[i * NPC : (i + 1) * NPC]),
            "xv": np.ascontiguousarray(v[i * NPC : (i + 1) * NPC]),
            "ss": np.ascontiguousarray(ss[i * NPC : (i + 1) * NPC]),
            "wstk": wstk,
            "bias1": b1,
            "bias2": b2,
        }
        for i in range(N_CORES)
    ]


def kernel(x, w, b):
    from concourse.bass_utils import run_bass_kernel_spmd

    x = np.asarray(x, dtype=np.float32)
    w = np.asarray(w, dtype=np.float32)
    b = np.asarray(b, dtype=np.float32)
    assert x.shape == (N, C, W) and w.shape == (F, C, WW) and b.shape == (F,)

    nc = _get_nc()
    in_maps = _make_in_maps(x, w, b)
    res = run_bass_kernel_spmd(nc, in_maps, core_ids=list(range(N_CORES)))
    out = np.concatenate([np.asarray(r["out"]) for r in res.results], axis=0)
    return out.astype(np.float32)
